# revision 1
# baseline (speedup 1.0000x reference)
"""CrossAttentionFusion kernel for Trainium2 (8 NeuronCores, Bass/Tile).

Computation (matches the reference nn.Module):
  image_proj = relu(BN(1x1conv(image_features, image_w)))   # (B,128,H,W)
  lidar_proj = relu(BN(1x1conv(lidar_features, lidar_w)))   # (B,128,H,W)
  per (batch, 2048-pixel chunk): q = image_proj, k = v = lidar_proj
  attn_out = softmax(q k^T / sqrt(128)) @ k
  out = w0 * image_proj + w1 * attn_out,  w = softmax(modality_weights)

Sharding: the 16 independent (batch, chunk) attention problems are
distributed 2-per-core across 8 cores; each core also computes the
projections for its own pixels.  Host gathers the 8 outputs.

Per-core kernel layout notes:
  - Projections are computed channel-major ([C=128 partitions, pixels]),
    which is the natural layout of both DRAM input and output.
  - Scores are computed k-major: sT[kpix, q] = (kT_slice)^T-style matmul
    with both operands channel-major.  exp() on the scalar engine.
  - AV uses transposed-K tiles: outT[c, q] += Kpix_i^T @ ET_i (N=512).
  - softmax denominator: S = sum_i ET_i (vector engine), then
    ones^T @ S broadcast-sums it across partitions on the PE.
  - w0 is folded into the image BN affine (relu(w0*x) = w0*relu(x));
    the exp scale compensates with 1/(w0*sqrt(C)).
"""

import math
import os
import sys
from contextlib import ExitStack

import numpy as np

sys.path.insert(0, "/opt/trn_rl_repo")

import concourse.bass as bass  # noqa: E402
import concourse.tile as tile  # noqa: E402
from concourse import bacc, mybir  # noqa: E402
from concourse.bass import ds, ts  # noqa: E402
from concourse.bass_utils import run_bass_kernel_spmd  # noqa: E402

F32 = mybir.dt.float32
F32R = mybir.dt.float32r

B, CL, CI, CO = 2, 256, 512, 128
H = W = 128
P = H * W                    # 16384 pixels per batch
CHUNK = 2048                 # attention chunk (pixels)
NCH = P // CHUNK             # 8 chunks per batch
NCORES = 8
UPC = (B * NCH) // NCORES    # units (b,chunk) per core = 2
EPS = 1e-5
QB = 1024                    # q-block width (2 matmul halves of 512)
NQB = CHUNK // QB            # 2
KSL = CHUNK // 128           # 16 k-pixel slices per chunk
NCI_IMG = CI // 128          # 4 contraction slices for image proj
NCI_LID = CL // 128          # 2 for lidar proj

_PROGRAM = None              # compiled Bass program, built once per process
LAST_RESULTS = None          # BassKernelResults of the last kernel() call


def _build_program():
    nc = bacc.Bacc("TRN2", target_bir_lowering=False, debug=False,
                   num_devices=NCORES)

    # Per-core DRAM inputs (pre-sharded on host).
    ximg = nc.dram_tensor("ximg", [UPC, NCI_IMG, 128, CHUNK], F32R,
                          kind="ExternalInput").ap()
    xlid = nc.dram_tensor("xlid", [UPC, NCI_LID, 128, CHUNK], F32R,
                          kind="ExternalInput").ap()
    wimg = nc.dram_tensor("wimg", [NCI_IMG, 128, CO], F32R,
                          kind="ExternalInput").ap()
    wlid = nc.dram_tensor("wlid", [NCI_LID, 128, CO], F32R,
                          kind="ExternalInput").ap()
    img_scale = nc.dram_tensor("img_scale", [CO, 1], F32, kind="ExternalInput").ap()
    img_bias = nc.dram_tensor("img_bias", [CO, 1], F32, kind="ExternalInput").ap()
    lid_scale = nc.dram_tensor("lid_scale", [CO, 1], F32, kind="ExternalInput").ap()
    lid_bias = nc.dram_tensor("lid_bias", [CO, 1], F32, kind="ExternalInput").ap()
    escale = nc.dram_tensor("escale", [128, 1], F32, kind="ExternalInput").ap()
    w1t = nc.dram_tensor("w1t", [128, 1], F32, kind="ExternalInput").ap()
    ident = nc.dram_tensor("ident", [128, 128], F32R, kind="ExternalInput").ap()
    ones_m = nc.dram_tensor("ones_m", [128, 128], F32R, kind="ExternalInput").ap()
    y = nc.dram_tensor("y", [UPC, CO, CHUNK], F32, kind="ExternalOutput").ap()

    with tile.TileContext(nc) as tc, ExitStack() as ctx:
        const = ctx.enter_context(tc.tile_pool(name="const", bufs=1))
        xi_pool = ctx.enter_context(tc.tile_pool(name="xi", bufs=6))
        xl_pool = ctx.enter_context(tc.tile_pool(name="xl", bufs=2 * NCI_LID))
        proj_pool = ctx.enter_context(tc.tile_pool(name="proj", bufs=2))
        kp_pool = ctx.enter_context(tc.tile_pool(name="kp", bufs=4))
        et_pool = ctx.enter_context(tc.tile_pool(name="et", bufs=4))
        misc_pool = ctx.enter_context(tc.tile_pool(name="misc", bufs=2))
        res_pool = ctx.enter_context(tc.tile_pool(name="res", bufs=2))
        # PSUM: mm 2x[128,1024](4 banks) + av 1x(2) + lb 1x(2) = 8 banks.
        # Transposes borrow mm slots between attention phases.
        mm_psum = ctx.enter_context(tc.tile_pool(name="mmps", bufs=2, space="PSUM"))
        av_psum = ctx.enter_context(tc.tile_pool(name="avps", bufs=1, space="PSUM"))
        lb_psum = ctx.enter_context(tc.tile_pool(name="lbps", bufs=1, space="PSUM"))

        # constants
        wimg_t = const.tile([128, NCI_IMG * CO], F32R)
        for ci in range(NCI_IMG):
            nc.sync.dma_start(wimg_t[:, ts(ci, CO)], wimg[ci])
        wlid_t = const.tile([128, NCI_LID * CO], F32R)
        for ci in range(NCI_LID):
            nc.sync.dma_start(wlid_t[:, ts(ci, CO)], wlid[ci])
        img_s = const.tile([128, 1], F32)
        nc.sync.dma_start(img_s[:], img_scale)
        img_b = const.tile([128, 1], F32)
        nc.sync.dma_start(img_b[:], img_bias)
        lid_s = const.tile([128, 1], F32)
        nc.sync.dma_start(lid_s[:], lid_scale)
        lid_b = const.tile([128, 1], F32)
        nc.sync.dma_start(lid_b[:], lid_bias)
        esc = const.tile([128, 1], F32)
        nc.sync.dma_start(esc[:], escale)
        w1s = const.tile([128, 1], F32)
        nc.sync.dma_start(w1s[:], w1t)
        ident_t = const.tile([128, 128], F32R)
        nc.sync.dma_start(ident_t[:], ident)
        ones_t = const.tile([128, 128], F32R)
        nc.sync.dma_start(ones_t[:], ones_m)

        for u in range(UPC):
            # ---- load unit inputs ----
            xi = []
            for ci in range(NCI_IMG):
                t = xi_pool.tile([128, CHUNK], F32R, name=f"xi_{u}_{ci}", tag="xi")
                for hh in range(2):
                    nc.sync.dma_start(t[:, ts(hh, QB)], ximg[u, ci, :, ts(hh, QB)])
                xi.append(t)
            xl = []
            for ci in range(NCI_LID):
                t = xl_pool.tile([128, CHUNK], F32R, name=f"xl_{u}_{ci}", tag="xl")
                for hh in range(2):
                    nc.sync.dma_start(t[:, ts(hh, QB)], xlid[u, ci, :, ts(hh, QB)])
                xl.append(t)

            # ---- projections (channel-major), QB-wide PSUM, halves of 512 ----
            qT = proj_pool.tile([128, CHUNK], F32R, name=f"qT_{u}", tag="qT")
            kT = proj_pool.tile([128, CHUNK], F32R, name=f"kT_{u}", tag="kT")
            for qb in range(NQB):
                ps = mm_psum.tile([128, QB], F32, name=f"psi_{u}_{qb}", tag="mm")
                for h in range(QB // 512):
                    for ci in range(NCI_IMG):
                        nc.tensor.matmul(ps[:, ts(h, 512)], wimg_t[:, ts(ci, CO)],
                                         xi[ci][:, ds(qb * QB + h * 512, 512)],
                                         start=(ci == 0), stop=(ci == NCI_IMG - 1))
                nc.scalar.activation(qT[:, ts(qb, QB)], ps[:],
                                     mybir.ActivationFunctionType.Relu,
                                     bias=img_b[:], scale=img_s[:])
                ps2 = mm_psum.tile([128, QB], F32, name=f"psl_{u}_{qb}", tag="mm")
                for h in range(QB // 512):
                    for ci in range(NCI_LID):
                        nc.tensor.matmul(ps2[:, ts(h, 512)], wlid_t[:, ts(ci, CO)],
                                         xl[ci][:, ds(qb * QB + h * 512, 512)],
                                         start=(ci == 0), stop=(ci == NCI_LID - 1))
                nc.scalar.activation(kT[:, ts(qb, QB)], ps2[:],
                                     mybir.ActivationFunctionType.Relu,
                                     bias=lid_b[:], scale=lid_s[:])

            # ---- transpose K to pixel-major tiles (8 transposes per wide
            # PSUM tile borrowed from the mm pool, one wide DVE copy) ----
            kpw = []
            for g in range(KSL // 8):
                pt = mm_psum.tile([128, QB], F32R, name=f"pt_{u}_{g}", tag="mm")
                for k in range(8):
                    nc.tensor.transpose(pt[:, ts(k, 128)],
                                        kT[:, ts(g * 8 + k, 128)], ident_t[:])
                kpt = kp_pool.tile([128, 8 * 128], F32R,
                                   name=f"kp_{u}_{g}", tag="kp")
                nc.vector.tensor_copy(kpt[:], pt[:, : 8 * 128])
                kpw.append(kpt)

            # ---- attention, one q-block at a time ----
            res_u = res_pool.tile([128, CHUNK], F32, name=f"res_{u}", tag="res")
            LOOKAHEAD = 2  # AV/denominator matmuls lag scores so the in-order
            #                PE queue never stalls waiting on ACT-engine exp
            for qb in range(NQB):
                po = av_psum.tile([128, QB], F32, name=f"po_{u}_{qb}", tag="av")
                S = et_pool.tile([128, QB], F32R, name=f"S_{u}_{qb}", tag="S",
                                 bufs=2)
                ets = [None] * KSL
                for i in range(KSL + LOOKAHEAD):
                    if i < KSL:
                        ps = mm_psum.tile([128, QB], F32,
                                          name=f"pss_{u}_{qb}_{i}", tag="mm")
                        for h in range(QB // 512):
                            nc.tensor.matmul(ps[:, ts(h, 512)], kT[:, ts(i, 128)],
                                             qT[:, ds(qb * QB + h * 512, 512)],
                                             start=True, stop=True)
                        et = et_pool.tile([128, QB], F32R,
                                          name=f"et_{u}_{qb}_{i}", tag="et")
                        nc.scalar.activation(et[:], ps[:],
                                             mybir.ActivationFunctionType.Exp,
                                             scale=esc[:])
                        ets[i] = et
                        if i == 0:
                            nc.vector.tensor_copy(S[:], et[:])
                        else:
                            nc.vector.tensor_add(S[:], S[:], et[:])
                    j = i - LOOKAHEAD
                    if j >= 0:
                        kslice = kpw[j // 8][:, ts(j % 8, 128)]
                        for h in range(QB // 512):
                            nc.tensor.matmul(po[:, ts(h, 512)], kslice,
                                             ets[j][:, ts(h, 512)],
                                             start=(j == 0), stop=(j == KSL - 1))
                pl = lb_psum.tile([128, QB], F32, name=f"pl_{u}_{qb}", tag="lb")
                for h in range(QB // 512):
                    nc.tensor.matmul(pl[:, ts(h, 512)], ones_t[:],
                                     S[:, ts(h, 512)], start=True, stop=True)
                linv = misc_pool.tile([128, QB], F32, name=f"linv_{u}_{qb}",
                                      tag="linv")
                nc.vector.reciprocal_approx_fast(linv[:], pl[:])
                tmp = misc_pool.tile([128, QB], F32, name=f"tmp_{u}_{qb}",
                                     tag="tmp")
                nc.vector.tensor_mul(tmp[:], po[:], linv[:])
                # res = w1 * (attn_out) + w0*image_proj   (qT already has w0)
                nc.vector.scalar_tensor_tensor(res_u[:, ts(qb, QB)], tmp[:],
                                               w1s[:], qT[:, ts(qb, QB)],
                                               op0=mybir.AluOpType.mult,
                                               op1=mybir.AluOpType.add)
            nc.sync.dma_start(y[u], res_u[:])

    nc.compile()
    return nc


def _shard_inputs(inputs):
    """Build the 8 per-core input maps from the full input dict."""
    mw = np.asarray(inputs["modality_weights"], np.float64)
    e = np.exp(mw - mw.max())
    w = (e / e.sum()).astype(np.float64)
    w0, w1 = float(w[0]), float(w[1])

    def bn_fold(gamma, beta, mean, var, mul):
        g = np.asarray(gamma, np.float64)
        b = np.asarray(beta, np.float64)
        m = np.asarray(mean, np.float64)
        v = np.asarray(var, np.float64)
        scale = g / np.sqrt(v + EPS) * mul
        bias = (b - m * g / np.sqrt(v + EPS)) * mul
        return (scale.astype(np.float32).reshape(CO, 1),
                bias.astype(np.float32).reshape(CO, 1))

    i_s, i_b = bn_fold(inputs["image_gamma"], inputs["image_beta"],
                       inputs["image_mean"], inputs["image_var"], w0)
    l_s, l_b = bn_fold(inputs["lidar_gamma"], inputs["lidar_beta"],
                       inputs["lidar_mean"], inputs["lidar_var"], 1.0)

    # weight slices, pre-transposed for lhsT ([cin_slice, cout])
    wi = np.ascontiguousarray(
        np.asarray(inputs["image_w"], np.float32).T.reshape(NCI_IMG, 128, CO))
    wl = np.ascontiguousarray(
        np.asarray(inputs["lidar_w"], np.float32).T.reshape(NCI_LID, 128, CO))

    esc = np.full((128, 1), 1.0 / (w0 * math.sqrt(CO)), np.float32)
    w1v = np.full((128, 1), w1, np.float32)
    ident = np.eye(128, dtype=np.float32)
    ones_m = np.ones((128, 128), np.float32)

    # full features reshaped to (B, nchunks, C, 2048)
    img = np.asarray(inputs["image_features"], np.float32).reshape(B, CI, NCH, CHUNK)
    lid = np.asarray(inputs["lidar_features"], np.float32).reshape(B, CL, NCH, CHUNK)

    in_maps = []
    for core in range(NCORES):
        ximg = np.empty((UPC, NCI_IMG, 128, CHUNK), np.float32)
        xlid = np.empty((UPC, NCI_LID, 128, CHUNK), np.float32)
        for ul in range(UPC):
            un = core * UPC + ul
            b, c = un // NCH, un % NCH
            ximg[ul] = img[b, :, c, :].reshape(NCI_IMG, 128, CHUNK)
            xlid[ul] = lid[b, :, c, :].reshape(NCI_LID, 128, CHUNK)
        in_maps.append({
            "ximg": ximg, "xlid": xlid, "wimg": wi, "wlid": wl,
            "img_scale": i_s, "img_bias": i_b,
            "lid_scale": l_s, "lid_bias": l_b,
            "escale": esc, "w1t": w1v, "ident": ident, "ones_m": ones_m,
        })
    return in_maps


def kernel(**inputs) -> np.ndarray:
    global _PROGRAM, LAST_RESULTS
    if _PROGRAM is None:
        _PROGRAM = _build_program()
    nc = _PROGRAM

    in_maps = _shard_inputs(inputs)
    trace = os.environ.get("BASS_KERNEL_TRACE", "0") == "1"
    tmpdir = os.environ.get("BASS_KERNEL_TRACE_DIR") or None
    if tmpdir:
        os.makedirs(tmpdir, exist_ok=True)
    results = run_bass_kernel_spmd(nc, in_maps, core_ids=list(range(NCORES)),
                                   trace=trace, tmpdir=tmpdir)
    LAST_RESULTS = results

    out = np.empty((B, CO, H, W), np.float32)
    outv = out.reshape(B, CO, NCH, CHUNK)
    for core in range(NCORES):
        yc = results.results[core]["y"]
        for ul in range(UPC):
            un = core * UPC + ul
            b, c = un // NCH, un % NCH
            outv[b, :, c, :] = yc[ul]
    return out


if __name__ == "__main__":
    rng = np.random.default_rng(0)
    inputs = {
        "lidar_features": rng.standard_normal((B, CL, H, W), np.float32),
        "image_features": rng.standard_normal((B, CI, H, W), np.float32),
        "lidar_w": rng.standard_normal((CO, CL), np.float32) * np.sqrt(2.0 / CO),
        "lidar_gamma": np.ones(CO, np.float32),
        "lidar_beta": np.zeros(CO, np.float32),
        "lidar_mean": rng.standard_normal(CO).astype(np.float32) * 0.1,
        "lidar_var": rng.uniform(0.5, 1.5, CO).astype(np.float32),
        "image_w": rng.standard_normal((CO, CI), np.float32) * np.sqrt(2.0 / CO),
        "image_gamma": np.ones(CO, np.float32),
        "image_beta": np.zeros(CO, np.float32),
        "image_mean": rng.standard_normal(CO).astype(np.float32) * 0.1,
        "image_var": rng.uniform(0.5, 1.5, CO).astype(np.float32),
        "modality_weights": np.ones(2, np.float32),
    }
    out = kernel(**inputs)
    print("kernel out:", out.shape, out.dtype, float(np.abs(out).mean()))



# revision 3
# speedup vs baseline: 1.1520x; 1.1520x over previous
"""CrossAttentionFusion kernel for Trainium2 (8 NeuronCores, Bass/Tile).

Computation (matches the reference nn.Module):
  image_proj = relu(BN(1x1conv(image_features, image_w)))   # (B,128,H,W)
  lidar_proj = relu(BN(1x1conv(lidar_features, lidar_w)))   # (B,128,H,W)
  per (batch, 2048-pixel chunk): q = image_proj, k = v = lidar_proj
  attn_out = softmax(q k^T / sqrt(128)) @ k
  out = w0 * image_proj + w1 * attn_out,  w = softmax(modality_weights)

Sharding: the 16 independent (batch, chunk) attention problems are
distributed 2-per-core across 8 cores; each core also computes the
projections for its own pixels.  Host gathers the 8 outputs.

Per-core kernel layout notes (bf16 pipeline):
  - All matmul operands are bf16 (weights, features, qT/kT, kp, et);
    PSUM accumulation stays fp32.  Host converts inputs to bf16.
  - Projections are computed channel-major ([C=128 partitions, pixels]).
  - Scores are computed k-major: ps[kslice, q] with both operands
    channel-major; exp() on the scalar engine writes bf16.
  - AV uses transposed-K tiles: po[c, q] += Kpix_i^T @ ET_i.
  - softmax denominator: binary-tree bf16 adds of the 16 ET tiles on
    the vector engine (S), then (ones/w1)^T @ S broadcast-sums across
    partitions on the PE; linv = 1/pl already carries w1.
  - w0 is folded into the image BN affine (relu(w0*x) = w0*relu(x));
    the exp scale compensates with 1/(w0*sqrt(C)).
  - Output written per-qb as bf16; host casts back to fp32.
"""

import math
import os
import sys
from contextlib import ExitStack

import ml_dtypes
import numpy as np

sys.path.insert(0, "/opt/trn_rl_repo")

import concourse.bass as bass  # noqa: E402
import concourse.tile as tile  # noqa: E402
from concourse import bacc, mybir  # noqa: E402
from concourse.bass import ds, ts  # noqa: E402
from concourse.bass_utils import run_bass_kernel_spmd  # noqa: E402

F32 = mybir.dt.float32
BF16 = mybir.dt.bfloat16
NPBF16 = ml_dtypes.bfloat16

B, CL, CI, CO = 2, 256, 512, 128
H = W = 128
P = H * W                    # 16384 pixels per batch
CHUNK = 2048                 # attention chunk (pixels)
NCH = P // CHUNK             # 8 chunks per batch
NCORES = 8
UPC = (B * NCH) // NCORES    # units (b,chunk) per core = 2
EPS = 1e-5
QB = 1024                    # q-block width (2 matmul halves of 512)
NQB = CHUNK // QB            # 2
KSL = CHUNK // 128           # 16 k-pixel slices per chunk
NCI_IMG = CI // 128          # 4 contraction slices for image proj
NCI_LID = CL // 128          # 2 for lidar proj

_PROGRAM = None              # compiled Bass program, built once per process
LAST_RESULTS = None          # BassKernelResults of the last kernel() call


def _build_program():
    nc = bacc.Bacc("TRN2", target_bir_lowering=False, debug=False,
                   num_devices=NCORES)

    # Per-core DRAM inputs (pre-sharded, bf16 on host).
    ximg = nc.dram_tensor("ximg", [UPC, NCI_IMG, 128, CHUNK], BF16,
                          kind="ExternalInput").ap()
    xlid = nc.dram_tensor("xlid", [UPC, NCI_LID, 128, CHUNK], BF16,
                          kind="ExternalInput").ap()
    wimg = nc.dram_tensor("wimg", [NCI_IMG, 128, CO], BF16,
                          kind="ExternalInput").ap()
    wlid = nc.dram_tensor("wlid", [NCI_LID, 128, CO], BF16,
                          kind="ExternalInput").ap()
    img_scale = nc.dram_tensor("img_scale", [CO, 1], F32, kind="ExternalInput").ap()
    img_bias = nc.dram_tensor("img_bias", [CO, 1], F32, kind="ExternalInput").ap()
    lid_scale = nc.dram_tensor("lid_scale", [CO, 1], F32, kind="ExternalInput").ap()
    lid_bias = nc.dram_tensor("lid_bias", [CO, 1], F32, kind="ExternalInput").ap()
    escale = nc.dram_tensor("escale", [128, 1], F32, kind="ExternalInput").ap()
    ident = nc.dram_tensor("ident", [128, 128], BF16, kind="ExternalInput").ap()
    ones_m = nc.dram_tensor("ones_m", [128, 128], BF16, kind="ExternalInput").ap()
    y = nc.dram_tensor("y", [UPC, NQB, CO, QB], BF16, kind="ExternalOutput").ap()

    with tile.TileContext(nc) as tc, ExitStack() as ctx:
        const = ctx.enter_context(tc.tile_pool(name="const", bufs=1))
        xi_pool = ctx.enter_context(tc.tile_pool(name="xi", bufs=2 * NCI_IMG))
        xl_pool = ctx.enter_context(tc.tile_pool(name="xl", bufs=2 * NCI_LID))
        proj_pool = ctx.enter_context(tc.tile_pool(name="proj", bufs=4))
        kp_pool = ctx.enter_context(tc.tile_pool(name="kp", bufs=4))
        et_pool = ctx.enter_context(tc.tile_pool(name="et", bufs=6))
        tree_pool = ctx.enter_context(tc.tile_pool(name="tree", bufs=10))
        misc_pool = ctx.enter_context(tc.tile_pool(name="misc", bufs=2))
        res_pool = ctx.enter_context(tc.tile_pool(name="res", bufs=2))
        # PSUM (8 banks of [128, 2KB]):
        #   mm 2x[128,1024]f32 (4 banks) - scores, double buffered; also
        #     borrowed for proj (img1/lid1), transposes (bf16) and lb
        #   av 1x[128,1024]f32 (2 banks) - AV accumulation
        #   sp 1x[128,1024]f32 (2 banks) - proj img0/lid0, transpose 0
        mm_psum = ctx.enter_context(tc.tile_pool(name="mmps", bufs=2, space="PSUM"))
        av_psum = ctx.enter_context(tc.tile_pool(name="avps", bufs=1, space="PSUM"))
        sp_psum = ctx.enter_context(tc.tile_pool(name="spps", bufs=1, space="PSUM"))

        # constants
        wimg_t = const.tile([128, NCI_IMG * CO], BF16)
        for ci in range(NCI_IMG):
            nc.sync.dma_start(wimg_t[:, ts(ci, CO)], wimg[ci])
        wlid_t = const.tile([128, NCI_LID * CO], BF16)
        for ci in range(NCI_LID):
            nc.sync.dma_start(wlid_t[:, ts(ci, CO)], wlid[ci])
        img_s = const.tile([128, 1], F32)
        nc.sync.dma_start(img_s[:], img_scale)
        img_b = const.tile([128, 1], F32)
        nc.sync.dma_start(img_b[:], img_bias)
        lid_s = const.tile([128, 1], F32)
        nc.sync.dma_start(lid_s[:], lid_scale)
        lid_b = const.tile([128, 1], F32)
        nc.sync.dma_start(lid_b[:], lid_bias)
        esc = const.tile([128, 1], F32)
        nc.sync.dma_start(esc[:], escale)
        ident_t = const.tile([128, 128], BF16)
        nc.sync.dma_start(ident_t[:], ident)
        ones_t = const.tile([128, 128], BF16)
        nc.sync.dma_start(ones_t[:], ones_m)

        def proj_group(ps, w_t, x_tiles, nci, col0):
            """Accumulate one [128, QB] projection block into PSUM ps.
            ci-outer / half-inner so consecutive matmuls share lhsT."""
            for ci in range(nci):
                for h in range(QB // 512):
                    nc.tensor.matmul(ps[:, ts(h, 512)], w_t[:, ts(ci, CO)],
                                     x_tiles[ci][:, ds(col0 + h * 512, 512)],
                                     start=(ci == 0), stop=(ci == nci - 1))

        for u in range(UPC):
            # ---- load unit inputs (halves so compute can start early) ----
            xi = []
            for ci in range(NCI_IMG):
                t = xi_pool.tile([128, CHUNK], BF16, name=f"xi_{u}_{ci}", tag="xi")
                for hh in range(2):
                    nc.sync.dma_start(t[:, ts(hh, QB)], ximg[u, ci, :, ts(hh, QB)])
                xi.append(t)
            xl = []
            for ci in range(NCI_LID):
                t = xl_pool.tile([128, CHUNK], BF16, name=f"xl_{u}_{ci}", tag="xl")
                for hh in range(2):
                    nc.sync.dma_start(t[:, ts(hh, QB)], xlid[u, ci, :, ts(hh, QB)])
                xl.append(t)

            # ---- projections (channel-major) ----
            qT = proj_pool.tile([128, CHUNK], BF16, name=f"qT_{u}", tag="qT")
            kT = proj_pool.tile([128, CHUNK], BF16, name=f"kT_{u}", tag="kT")
            for qb in range(NQB):
                pool = sp_psum if qb == 0 else mm_psum
                ps = pool.tile([128, QB], F32, name=f"psi_{u}_{qb}", tag="ps")
                proj_group(ps, wimg_t, xi, NCI_IMG, qb * QB)
                nc.scalar.activation(qT[:, ts(qb, QB)], ps[:],
                                     mybir.ActivationFunctionType.Relu,
                                     bias=img_b[:], scale=img_s[:])
            for qb in range(NQB):
                pool = sp_psum if qb == 0 else mm_psum
                ps2 = pool.tile([128, QB], F32, name=f"psl_{u}_{qb}", tag="ps")
                proj_group(ps2, wlid_t, xl, NCI_LID, qb * QB)
                nc.scalar.activation(kT[:, ts(qb, QB)], ps2[:],
                                     mybir.ActivationFunctionType.Relu,
                                     bias=lid_b[:], scale=lid_s[:])

            # ---- transpose K to pixel-major tiles (bf16 PSUM transpose,
            # one 2x-eligible DVE copy per 8-slice group) ----
            kpw = []
            for g in range(KSL // 8):
                pool = sp_psum if g == 0 else mm_psum
                pt = pool.tile([128, 8 * 128], BF16, name=f"pt_{u}_{g}", tag="ps")
                for k in range(8):
                    nc.tensor.transpose(pt[:, ts(k, 128)],
                                        kT[:, ts(g * 8 + k, 128)], ident_t[:])
                kpt = kp_pool.tile([128, 8 * 128], BF16,
                                   name=f"kp_{u}_{g}", tag="kp")
                nc.vector.tensor_copy(kpt[:], pt[:])
                kpw.append(kpt)

            # ---- attention, one q-block at a time ----
            LOOKAHEAD = 2  # AV matmuls lag scores so the in-order PE queue
            #                never stalls waiting on ACT-engine exp
            for qb in range(NQB):
                po = av_psum.tile([128, QB], F32, name=f"po_{u}_{qb}", tag="av")
                ets = [None] * KSL
                lvl1 = [None] * (KSL // 2)
                lvl2 = [None] * (KSL // 4)
                lvl3 = [None] * (KSL // 8)
                for i in range(KSL + LOOKAHEAD):
                    if i < KSL:
                        ps = mm_psum.tile([128, QB], F32,
                                          name=f"pss_{u}_{qb}_{i}", tag="ps")
                        for h in range(QB // 512):
                            nc.tensor.matmul(ps[:, ts(h, 512)], kT[:, ts(i, 128)],
                                             qT[:, ds(qb * QB + h * 512, 512)],
                                             start=True, stop=True)
                        et = et_pool.tile([128, QB], BF16,
                                          name=f"et_{u}_{qb}_{i}", tag="et")
                        nc.scalar.activation(et[:], ps[:],
                                             mybir.ActivationFunctionType.Exp,
                                             scale=esc[:])
                        ets[i] = et
                        # binary-tree accumulation of S (all-bf16 adds)
                        if i % 2 == 1:
                            t = tree_pool.tile([128, QB], BF16,
                                               name=f"t1_{u}_{qb}_{i}", tag="t1")
                            nc.vector.tensor_add(t[:], ets[i - 1][:], et[:])
                            lvl1[i // 2] = t
                            if i % 4 == 3:
                                t2 = tree_pool.tile([128, QB], BF16,
                                                    name=f"t2_{u}_{qb}_{i}",
                                                    tag="t1")
                                nc.vector.tensor_add(t2[:], lvl1[i // 2 - 1][:],
                                                     t[:])
                                lvl2[i // 4] = t2
                                if i % 8 == 7:
                                    t3 = tree_pool.tile([128, QB], BF16,
                                                        name=f"t3_{u}_{qb}_{i}",
                                                        tag="t1")
                                    nc.vector.tensor_add(t3[:],
                                                         lvl2[i // 4 - 1][:],
                                                         t2[:])
                                    lvl3[i // 8] = t3
                    j = i - LOOKAHEAD
                    if j >= 0:
                        kslice = kpw[j // 8][:, ts(j % 8, 128)]
                        for h in range(QB // 512):
                            nc.tensor.matmul(po[:, ts(h, 512)], kslice,
                                             ets[j][:, ts(h, 512)],
                                             start=(j == 0), stop=(j == KSL - 1))
                S = tree_pool.tile([128, QB], BF16, name=f"S_{u}_{qb}", tag="t1")
                nc.vector.tensor_add(S[:], lvl3[0][:], lvl3[1][:])
                # denominator broadcast across partitions; ones carry 1/w1
                pl = mm_psum.tile([128, QB], F32, name=f"pl_{u}_{qb}", tag="ps")
                for h in range(QB // 512):
                    nc.tensor.matmul(pl[:, ts(h, 512)], ones_t[:],
                                     S[:, ts(h, 512)], start=True, stop=True)
                linv = misc_pool.tile([128, QB], F32, name=f"linv_{u}_{qb}",
                                      tag="linv")
                nc.vector.reciprocal_approx_fast(linv[:], pl[:])
                tmp = misc_pool.tile([128, QB], F32, name=f"tmp_{u}_{qb}",
                                     tag="tmp")
                nc.vector.tensor_mul(tmp[:], po[:], linv[:])
                res = res_pool.tile([128, QB], BF16, name=f"res_{u}_{qb}",
                                    tag="res")
                nc.vector.tensor_add(res[:], tmp[:], qT[:, ts(qb, QB)])
                nc.sync.dma_start(y[u, qb], res[:])

    nc.compile()
    return nc


def _shard_inputs(inputs):
    """Build the 8 per-core input maps from the full input dict."""
    mw = np.asarray(inputs["modality_weights"], np.float64)
    e = np.exp(mw - mw.max())
    w = (e / e.sum()).astype(np.float64)
    w0, w1 = float(w[0]), float(w[1])

    def bn_fold(gamma, beta, mean, var, mul):
        g = np.asarray(gamma, np.float64)
        b = np.asarray(beta, np.float64)
        m = np.asarray(mean, np.float64)
        v = np.asarray(var, np.float64)
        scale = g / np.sqrt(v + EPS) * mul
        bias = (b - m * g / np.sqrt(v + EPS)) * mul
        return (scale.astype(np.float32).reshape(CO, 1),
                bias.astype(np.float32).reshape(CO, 1))

    i_s, i_b = bn_fold(inputs["image_gamma"], inputs["image_beta"],
                       inputs["image_mean"], inputs["image_var"], w0)
    l_s, l_b = bn_fold(inputs["lidar_gamma"], inputs["lidar_beta"],
                       inputs["lidar_mean"], inputs["lidar_var"], 1.0)

    # weight slices, pre-transposed for lhsT ([cin_slice, cout])
    wi = np.ascontiguousarray(
        np.asarray(inputs["image_w"], np.float32).T.reshape(NCI_IMG, 128, CO)
    ).astype(NPBF16)
    wl = np.ascontiguousarray(
        np.asarray(inputs["lidar_w"], np.float32).T.reshape(NCI_LID, 128, CO)
    ).astype(NPBF16)

    esc = np.full((128, 1), 1.0 / (w0 * math.sqrt(CO)), np.float32)
    ident = np.eye(128, dtype=np.float32).astype(NPBF16)
    # ones carry 1/w1 so linv = 1/pl = w1/denominator
    ones_m = np.full((128, 128), 1.0 / w1, np.float32).astype(NPBF16)

    # full features reshaped to (B, nchunks, C, 2048), bf16
    img = np.asarray(inputs["image_features"], np.float32).reshape(
        B, CI, NCH, CHUNK).astype(NPBF16)
    lid = np.asarray(inputs["lidar_features"], np.float32).reshape(
        B, CL, NCH, CHUNK).astype(NPBF16)

    in_maps = []
    for core in range(NCORES):
        ximg = np.empty((UPC, NCI_IMG, 128, CHUNK), NPBF16)
        xlid = np.empty((UPC, NCI_LID, 128, CHUNK), NPBF16)
        for ul in range(UPC):
            un = core * UPC + ul
            b, c = un // NCH, un % NCH
            ximg[ul] = img[b, :, c, :].reshape(NCI_IMG, 128, CHUNK)
            xlid[ul] = lid[b, :, c, :].reshape(NCI_LID, 128, CHUNK)
        in_maps.append({
            "ximg": ximg, "xlid": xlid, "wimg": wi, "wlid": wl,
            "img_scale": i_s, "img_bias": i_b,
            "lid_scale": l_s, "lid_bias": l_b,
            "escale": esc, "ident": ident, "ones_m": ones_m,
        })
    return in_maps


def kernel(**inputs) -> np.ndarray:
    global _PROGRAM, LAST_RESULTS
    if _PROGRAM is None:
        _PROGRAM = _build_program()
    nc = _PROGRAM

    in_maps = _shard_inputs(inputs)
    trace = os.environ.get("BASS_KERNEL_TRACE", "0") == "1"
    tmpdir = os.environ.get("BASS_KERNEL_TRACE_DIR") or None
    if tmpdir:
        os.makedirs(tmpdir, exist_ok=True)
    results = run_bass_kernel_spmd(nc, in_maps, core_ids=list(range(NCORES)),
                                   trace=trace, tmpdir=tmpdir)
    LAST_RESULTS = results

    out = np.empty((B, CO, H, W), np.float32)
    outv = out.reshape(B, CO, NCH, NQB, QB)
    for core in range(NCORES):
        yc = np.asarray(results.results[core]["y"], dtype=np.float32)
        for ul in range(UPC):
            un = core * UPC + ul
            b, c = un // NCH, un % NCH
            outv[b, :, c, :, :] = yc[ul].transpose(1, 0, 2)
    return out


if __name__ == "__main__":
    rng = np.random.default_rng(0)
    inputs = {
        "lidar_features": rng.standard_normal((B, CL, H, W), np.float32),
        "image_features": rng.standard_normal((B, CI, H, W), np.float32),
        "lidar_w": rng.standard_normal((CO, CL), np.float32) * np.sqrt(2.0 / CO),
        "lidar_gamma": np.ones(CO, np.float32),
        "lidar_beta": np.zeros(CO, np.float32),
        "lidar_mean": rng.standard_normal(CO).astype(np.float32) * 0.1,
        "lidar_var": rng.uniform(0.5, 1.5, CO).astype(np.float32),
        "image_w": rng.standard_normal((CO, CI), np.float32) * np.sqrt(2.0 / CO),
        "image_gamma": np.ones(CO, np.float32),
        "image_beta": np.zeros(CO, np.float32),
        "image_mean": rng.standard_normal(CO).astype(np.float32) * 0.1,
        "image_var": rng.uniform(0.5, 1.5, CO).astype(np.float32),
        "modality_weights": np.ones(2, np.float32),
    }
    out = kernel(**inputs)
    print("kernel out:", out.shape, out.dtype, float(np.abs(out).mean()))


# revision 10
# speedup vs baseline: 1.1861x; 1.0296x over previous
"""CrossAttentionFusion kernel for Trainium2 (8 NeuronCores, Bass/Tile).

Computation (matches the reference nn.Module):
  image_proj = relu(BN(1x1conv(image_features, image_w)))   # (B,128,H,W)
  lidar_proj = relu(BN(1x1conv(lidar_features, lidar_w)))   # (B,128,H,W)
  per (batch, 2048-pixel chunk): q = image_proj, k = v = lidar_proj
  attn_out = softmax(q k^T / sqrt(128)) @ k
  out = w0 * image_proj + w1 * attn_out,  w = softmax(modality_weights)

Sharding: the 16 independent (batch, chunk) attention problems are
distributed 2-per-core across 8 cores; each core also computes the
projections for its own pixels.  Host gathers the 8 outputs.

Per-core kernel layout notes (bf16 pipeline):
  - All matmul operands are bf16; PSUM accumulation stays fp32.  Host
    converts inputs to bf16 and folds the BN scale into the weights, so
    the projection epilogue is a single DVE tensor_scalar:
    max(psum + bias, 0) -> bf16.
  - Matmuls are 1024 wide (output spans two PSUM banks), halving the
    instruction + LDWEIGHTS count vs 512-wide halves.
  - Scores are computed k-major: ps[kslice, q]; exp() on the scalar
    engine writes bf16 (the scalar engine is the pace-setter: ~1.3us
    per 1024-wide exp, 64 of them).
  - AV uses transposed-K tiles: po[c, q] += Kpix_i^T @ ET_i, lagging
    LOOKAHEAD slices behind the score stream; the slice loop runs
    globally across both q-blocks so the PE never drains at block
    boundaries.
  - softmax denominator: binary-tree bf16 adds of the 16 ET tiles on
    the vector engine (S), then (ones/w1)^T @ S broadcast-sums across
    partitions on the PE; linv = 1/pl then already carries w1.
  - w0 is folded into the image BN affine (relu(w0*x) = w0*relu(x));
    the exp scale compensates with 1/(w0*sqrt(C)).
  - Output written per-qb as bf16; host casts back to fp32.
"""

import math
import os
import sys
from contextlib import ExitStack

import ml_dtypes
import numpy as np

sys.path.insert(0, "/opt/trn_rl_repo")

import concourse.bass as bass  # noqa: E402
import concourse.tile as tile  # noqa: E402
from concourse import bacc, mybir  # noqa: E402
from concourse.bass import ds, ts  # noqa: E402
from concourse.bass_utils import run_bass_kernel_spmd  # noqa: E402

F32 = mybir.dt.float32
BF16 = mybir.dt.bfloat16
NPBF16 = ml_dtypes.bfloat16

B, CL, CI, CO = 2, 256, 512, 128
H = W = 128
P = H * W                    # 16384 pixels per batch
CHUNK = 2048                 # attention chunk (pixels)
NCH = P // CHUNK             # 8 chunks per batch
NCORES = 8
UPC = (B * NCH) // NCORES    # units (b,chunk) per core = 2
EPS = 1e-5
QB = 1024                    # q-block width (one 2-bank PSUM matmul)
NQB = CHUNK // QB            # 2
KSL = CHUNK // 128           # 16 k-pixel slices per chunk
NSL = NQB * KSL              # 32 (qb, slice) score tiles per unit
NCI_IMG = CI // 128          # 4 contraction slices for image proj
NCI_LID = CL // 128          # 2 for lidar proj

_PROGRAM = None              # compiled Bass program, built once per process
LAST_RESULTS = None          # BassKernelResults of the last kernel() call


def _build_program():
    nc = bacc.Bacc("TRN2", target_bir_lowering=False, debug=False,
                   num_devices=NCORES)

    # Per-core DRAM inputs (pre-sharded, bf16 on host).
    ximg = nc.dram_tensor("ximg", [UPC, NCI_IMG, 128, CHUNK], BF16,
                          kind="ExternalInput").ap()
    xlid = nc.dram_tensor("xlid", [UPC, NCI_LID, 128, CHUNK], BF16,
                          kind="ExternalInput").ap()
    wimg = nc.dram_tensor("wimg", [NCI_IMG, 128, CO], BF16,
                          kind="ExternalInput").ap()
    wlid = nc.dram_tensor("wlid", [NCI_LID, 128, CO], BF16,
                          kind="ExternalInput").ap()
    img_bias = nc.dram_tensor("img_bias", [CO, 1], F32, kind="ExternalInput").ap()
    lid_bias = nc.dram_tensor("lid_bias", [CO, 1], F32, kind="ExternalInput").ap()
    escale = nc.dram_tensor("escale", [128, 1], F32, kind="ExternalInput").ap()
    ident = nc.dram_tensor("ident", [128, 128], BF16, kind="ExternalInput").ap()
    ones_m = nc.dram_tensor("ones_m", [128, 128], BF16, kind="ExternalInput").ap()
    y = nc.dram_tensor("y", [UPC, NQB, CO, QB], BF16, kind="ExternalOutput").ap()

    with tile.TileContext(nc) as tc, ExitStack() as ctx:
        const = ctx.enter_context(tc.tile_pool(name="const", bufs=1))
        xi_pool = ctx.enter_context(tc.tile_pool(name="xi", bufs=2 * NCI_IMG))
        xl_pool = ctx.enter_context(tc.tile_pool(name="xl", bufs=2 * NCI_LID))
        proj_pool = ctx.enter_context(tc.tile_pool(name="proj", bufs=4))
        kp_pool = ctx.enter_context(tc.tile_pool(name="kp", bufs=4))
        et_pool = ctx.enter_context(tc.tile_pool(name="et", bufs=8))
        tree_pool = ctx.enter_context(tc.tile_pool(name="tree", bufs=10))
        misc_pool = ctx.enter_context(tc.tile_pool(name="misc", bufs=2))
        res_pool = ctx.enter_context(tc.tile_pool(name="res", bufs=2))
        # PSUM (8 banks of [128, 2KB]):
        #   mm 2x[128,1024]f32 (4 banks) - scores, double buffered; also
        #     borrowed for proj, transposes (bf16) and the lb matmul
        #   av 2x[128,1024]f32 (4 banks) - AV accumulation, double buffered
        #     so the next q-block's AV can start before the previous blend
        mm_psum = ctx.enter_context(tc.tile_pool(name="mmps", bufs=2, space="PSUM"))
        av_psum = ctx.enter_context(tc.tile_pool(name="avps", bufs=2, space="PSUM"))

        # constants
        wimg_t = const.tile([128, NCI_IMG * CO], BF16)
        for ci in range(NCI_IMG):
            nc.sync.dma_start(wimg_t[:, ts(ci, CO)], wimg[ci])
        wlid_t = const.tile([128, NCI_LID * CO], BF16)
        for ci in range(NCI_LID):
            nc.sync.dma_start(wlid_t[:, ts(ci, CO)], wlid[ci])
        img_b = const.tile([128, 1], F32)
        nc.sync.dma_start(img_b[:], img_bias)
        lid_b = const.tile([128, 1], F32)
        nc.sync.dma_start(lid_b[:], lid_bias)
        esc = const.tile([128, 1], F32)
        nc.sync.dma_start(esc[:], escale)
        ident_t = const.tile([128, 128], BF16)
        nc.sync.dma_start(ident_t[:], ident)
        ones_t = const.tile([128, 128], BF16)
        nc.sync.dma_start(ones_t[:], ones_m)

        for u in range(UPC):
            # ---- load unit inputs, first halves of every tile first so the
            # first projection group can start as early as possible ----
            xi = [xi_pool.tile([128, CHUNK], BF16, name=f"xi_{u}_{ci}", tag="xi")
                  for ci in range(NCI_IMG)]
            xl = [xl_pool.tile([128, CHUNK], BF16, name=f"xl_{u}_{ci}", tag="xl")
                  for ci in range(NCI_LID)]
            for hh in range(2):
                for ci in range(NCI_IMG):
                    nc.sync.dma_start(xi[ci][:, ts(hh, QB)], ximg[u, ci, :, ts(hh, QB)])
                for ci in range(NCI_LID):
                    nc.sync.dma_start(xl[ci][:, ts(hh, QB)], xlid[u, ci, :, ts(hh, QB)])

            # ---- projections (channel-major, scale pre-folded in weights;
            # epilogue = max(psum + bias, 0) on the DVE) ----
            qT = proj_pool.tile([128, CHUNK], BF16, name=f"qT_{u}", tag="qT")
            kT = proj_pool.tile([128, CHUNK], BF16, name=f"kT_{u}", tag="kT")
            for qb in range(NQB):
                ps = mm_psum.tile([128, QB], F32, name=f"psi_{u}_{qb}", tag="ps")
                for ci in range(NCI_IMG):
                    for h in range(QB // 512):
                        nc.tensor.matmul(ps[:, ts(h, 512)], wimg_t[:, ts(ci, CO)],
                                         xi[ci][:, ds(qb * QB + h * 512, 512)],
                                         start=(ci == 0), stop=(ci == NCI_IMG - 1))
                nc.vector.tensor_scalar(qT[:, ts(qb, QB)], ps[:], img_b[:], 0.0,
                                        op0=mybir.AluOpType.add,
                                        op1=mybir.AluOpType.max)
            for qb in range(NQB):
                ps2 = mm_psum.tile([128, QB], F32, name=f"psl_{u}_{qb}", tag="ps")
                for ci in range(NCI_LID):
                    for h in range(QB // 512):
                        nc.tensor.matmul(ps2[:, ts(h, 512)], wlid_t[:, ts(ci, CO)],
                                         xl[ci][:, ds(qb * QB + h * 512, 512)],
                                         start=(ci == 0), stop=(ci == NCI_LID - 1))
                nc.vector.tensor_scalar(kT[:, ts(qb, QB)], ps2[:], lid_b[:], 0.0,
                                        op0=mybir.AluOpType.add,
                                        op1=mybir.AluOpType.max)

            # ---- transpose K to pixel-major tiles (bf16 PSUM transpose,
            # one 2x-eligible DVE copy per 8-slice group) ----
            kpw = []
            for g in range(KSL // 8):
                pt = mm_psum.tile([128, 8 * 128], BF16, name=f"pt_{u}_{g}",
                                  tag="ps")
                for k in range(8):
                    nc.tensor.transpose(pt[:, ts(k, 128)],
                                        kT[:, ts(g * 8 + k, 128)], ident_t[:])
                kpt = kp_pool.tile([128, 8 * 128], BF16,
                                   name=f"kp_{u}_{g}", tag="kp")
                nc.vector.tensor_copy(kpt[:], pt[:])
                kpw.append(kpt)

            # ---- attention: one global slice pipeline across both q-blocks.
            # scores/exp run LOOKAHEAD slices ahead of the AV matmuls so the
            # in-order PE queue never waits on the scalar engine. ----
            LOOKAHEAD = 3
            ets = [None] * NSL
            pos = [None] * NQB
            lvl1 = [None] * (KSL // 2)
            lvl2 = [None] * (KSL // 4)
            lvl3 = [None] * (KSL // 8)

            def qb_epilogue(qb, S):
                """Denominator broadcast + reciprocal + blend + store."""
                pl = mm_psum.tile([128, QB], F32, name=f"pl_{u}_{qb}", tag="ps")
                for h in range(QB // 512):
                    nc.tensor.matmul(pl[:, ts(h, 512)], ones_t[:],
                                     S[:, ts(h, 512)], start=True, stop=True)
                linv = misc_pool.tile([128, QB], F32, name=f"linv_{u}_{qb}",
                                      tag="linv")
                nc.vector.reciprocal_approx_fast(linv[:], pl[:])
                tmp = misc_pool.tile([128, QB], F32, name=f"tmp_{u}_{qb}",
                                     tag="tmp")
                nc.vector.tensor_mul(tmp[:], pos[qb][:], linv[:])
                res = res_pool.tile([128, QB], BF16, name=f"res_{u}_{qb}",
                                    tag="res")
                nc.vector.tensor_add(res[:], tmp[:], qT[:, ts(qb, QB)])
                nc.sync.dma_start(y[u, qb], res[:])

            for g in range(NSL + LOOKAHEAD):
                if g < NSL:
                    qb, i = divmod(g, KSL)
                    ps = mm_psum.tile([128, QB], F32,
                                      name=f"pss_{u}_{qb}_{i}", tag="ps")
                    for h in range(QB // 512):
                        nc.tensor.matmul(ps[:, ts(h, 512)], kT[:, ts(i, 128)],
                                         qT[:, ds(qb * QB + h * 512, 512)],
                                         start=True, stop=True)
                    et = et_pool.tile([128, QB], BF16,
                                      name=f"et_{u}_{qb}_{i}", tag="et")
                    nc.scalar.activation(et[:], ps[:],
                                         mybir.ActivationFunctionType.Exp,
                                         scale=esc[:])
                    ets[g] = et
                    # binary-tree accumulation of S (all-bf16 2x DVE adds)
                    if i % 2 == 1:
                        t = tree_pool.tile([128, QB], BF16,
                                           name=f"t1_{u}_{qb}_{i}", tag="t1")
                        nc.vector.tensor_add(t[:], ets[g - 1][:], et[:])
                        lvl1[i // 2] = t
                        if i % 4 == 3:
                            t2 = tree_pool.tile([128, QB], BF16,
                                                name=f"t2_{u}_{qb}_{i}",
                                                tag="t1")
                            nc.vector.tensor_add(t2[:], lvl1[i // 2 - 1][:],
                                                 t[:])
                            lvl2[i // 4] = t2
                            if i % 8 == 7:
                                t3 = tree_pool.tile([128, QB], BF16,
                                                    name=f"t3_{u}_{qb}_{i}",
                                                    tag="t1")
                                nc.vector.tensor_add(t3[:],
                                                     lvl2[i // 4 - 1][:],
                                                     t2[:])
                                lvl3[i // 8] = t3
                                if i == KSL - 1:
                                    S = tree_pool.tile([128, QB], BF16,
                                                       name=f"S_{u}_{qb}",
                                                       tag="t1")
                                    nc.vector.tensor_add(S[:], lvl3[0][:],
                                                         lvl3[1][:])
                                    lvl1 = [None] * (KSL // 2)
                                    lvl2 = [None] * (KSL // 4)
                                    lvl3 = [None] * (KSL // 8)
                                    pending_S = S
                j = g - LOOKAHEAD
                if j >= 0:
                    qbj, i = divmod(j, KSL)
                    if i == 0:
                        pos[qbj] = av_psum.tile([128, QB], F32,
                                                name=f"po_{u}_{qbj}", tag="av")
                    kslice = kpw[i // 8][:, ts(i % 8, 128)]
                    for h in range(QB // 512):
                        nc.tensor.matmul(pos[qbj][:, ts(h, 512)], kslice,
                                         ets[j][:, ts(h, 512)],
                                         start=(i == 0), stop=(i == KSL - 1))
                    if i == KSL - 1:
                        qb_epilogue(qbj, pending_S if qbj == NQB - 1 else S0)
                # stash qb0's S before the tree arrays are reused
                if g == KSL - 1:
                    S0 = pending_S

    nc.compile()
    return nc


def _shard_inputs(inputs):
    """Build the 8 per-core input maps from the full input dict."""
    mw = np.asarray(inputs["modality_weights"], np.float64)
    e = np.exp(mw - mw.max())
    w = (e / e.sum()).astype(np.float64)
    w0, w1 = float(w[0]), float(w[1])

    def bn_fold(gamma, beta, mean, var, mul):
        g = np.asarray(gamma, np.float64)
        b = np.asarray(beta, np.float64)
        m = np.asarray(mean, np.float64)
        v = np.asarray(var, np.float64)
        scale = g / np.sqrt(v + EPS) * mul
        bias = (b - m * g / np.sqrt(v + EPS)) * mul
        return scale, bias.astype(np.float32).reshape(CO, 1)

    i_s, i_b = bn_fold(inputs["image_gamma"], inputs["image_beta"],
                       inputs["image_mean"], inputs["image_var"], w0)
    l_s, l_b = bn_fold(inputs["lidar_gamma"], inputs["lidar_beta"],
                       inputs["lidar_mean"], inputs["lidar_var"], 1.0)

    # weight slices with the BN scale folded in, pre-transposed for lhsT
    # ([cin_slice, cout])
    wi = np.ascontiguousarray(
        (np.asarray(inputs["image_w"], np.float64) * i_s[:, None]).T.reshape(
            NCI_IMG, 128, CO)).astype(NPBF16)
    wl = np.ascontiguousarray(
        (np.asarray(inputs["lidar_w"], np.float64) * l_s[:, None]).T.reshape(
            NCI_LID, 128, CO)).astype(NPBF16)

    esc = np.full((128, 1), 1.0 / (w0 * math.sqrt(CO)), np.float32)
    ident = np.eye(128, dtype=np.float32).astype(NPBF16)
    # ones carry 1/w1 so linv = 1/pl = w1/denominator
    ones_m = np.full((128, 128), 1.0 / w1, np.float32).astype(NPBF16)

    # full features reshaped to (B, nchunks, C, 2048), bf16
    img = np.asarray(inputs["image_features"], np.float32).reshape(
        B, CI, NCH, CHUNK).astype(NPBF16)
    lid = np.asarray(inputs["lidar_features"], np.float32).reshape(
        B, CL, NCH, CHUNK).astype(NPBF16)

    in_maps = []
    for core in range(NCORES):
        ximg = np.empty((UPC, NCI_IMG, 128, CHUNK), NPBF16)
        xlid = np.empty((UPC, NCI_LID, 128, CHUNK), NPBF16)
        for ul in range(UPC):
            un = core * UPC + ul
            b, c = un // NCH, un % NCH
            ximg[ul] = img[b, :, c, :].reshape(NCI_IMG, 128, CHUNK)
            xlid[ul] = lid[b, :, c, :].reshape(NCI_LID, 128, CHUNK)
        in_maps.append({
            "ximg": ximg, "xlid": xlid, "wimg": wi, "wlid": wl,
            "img_bias": i_b, "lid_bias": l_b,
            "escale": esc, "ident": ident, "ones_m": ones_m,
        })
    return in_maps


def kernel(**inputs) -> np.ndarray:
    global _PROGRAM, LAST_RESULTS
    if _PROGRAM is None:
        _PROGRAM = _build_program()
    nc = _PROGRAM

    in_maps = _shard_inputs(inputs)
    trace = os.environ.get("BASS_KERNEL_TRACE", "0") == "1"
    tmpdir = os.environ.get("BASS_KERNEL_TRACE_DIR") or None
    if tmpdir:
        os.makedirs(tmpdir, exist_ok=True)
    results = run_bass_kernel_spmd(nc, in_maps, core_ids=list(range(NCORES)),
                                   trace=trace, tmpdir=tmpdir)
    LAST_RESULTS = results

    out = np.empty((B, CO, H, W), np.float32)
    outv = out.reshape(B, CO, NCH, NQB, QB)
    for core in range(NCORES):
        yc = np.asarray(results.results[core]["y"], dtype=np.float32)
        for ul in range(UPC):
            un = core * UPC + ul
            b, c = un // NCH, un % NCH
            outv[b, :, c, :, :] = yc[ul].transpose(1, 0, 2)
    return out


if __name__ == "__main__":
    rng = np.random.default_rng(0)
    inputs = {
        "lidar_features": rng.standard_normal((B, CL, H, W), np.float32),
        "image_features": rng.standard_normal((B, CI, H, W), np.float32),
        "lidar_w": rng.standard_normal((CO, CL), np.float32) * np.sqrt(2.0 / CO),
        "lidar_gamma": np.ones(CO, np.float32),
        "lidar_beta": np.zeros(CO, np.float32),
        "lidar_mean": rng.standard_normal(CO).astype(np.float32) * 0.1,
        "lidar_var": rng.uniform(0.5, 1.5, CO).astype(np.float32),
        "image_w": rng.standard_normal((CO, CI), np.float32) * np.sqrt(2.0 / CO),
        "image_gamma": np.ones(CO, np.float32),
        "image_beta": np.zeros(CO, np.float32),
        "image_mean": rng.standard_normal(CO).astype(np.float32) * 0.1,
        "image_var": rng.uniform(0.5, 1.5, CO).astype(np.float32),
        "modality_weights": np.ones(2, np.float32),
    }
    out = kernel(**inputs)
    print("kernel out:", out.shape, out.dtype, float(np.abs(out).mean()))


# revision 12
# speedup vs baseline: 1.2342x; 1.0406x over previous
"""CrossAttentionFusion kernel for Trainium2 (8 NeuronCores, Bass/Tile).

Computation (matches the reference nn.Module):
  image_proj = relu(BN(1x1conv(image_features, image_w)))   # (B,128,H,W)
  lidar_proj = relu(BN(1x1conv(lidar_features, lidar_w)))   # (B,128,H,W)
  per (batch, 2048-pixel chunk): q = image_proj, k = v = lidar_proj
  attn_out = softmax(q k^T / sqrt(128)) @ k
  out = w0 * image_proj + w1 * attn_out,  w = softmax(modality_weights)

Sharding: the 16 independent (batch, chunk) attention problems are
distributed 2-per-core across 8 cores; each core also computes the
projections for its own pixels.  Host gathers the 8 outputs.

Per-core kernel layout notes (bf16 pipeline):
  - All matmul operands are bf16; PSUM accumulation stays fp32.  Host
    converts inputs to bf16 and folds the BN scale into the weights, so
    the projection epilogue is a single DVE tensor_scalar:
    max(psum + bias, 0) -> bf16.
  - Matmuls are 1024 wide (output spans two PSUM banks), halving the
    instruction + LDWEIGHTS count vs 512-wide halves.
  - Scores are computed k-major: ps[kslice, q]; exp() on the scalar
    engine writes bf16 (the scalar engine is the pace-setter: ~1.3us
    per 1024-wide exp, 64 of them).
  - AV uses transposed-K tiles: po[c, q] += Kpix_i^T @ ET_i, lagging
    LOOKAHEAD slices behind the score stream; the slice loop runs
    globally across both q-blocks so the PE never drains at block
    boundaries.
  - softmax denominator: binary-tree bf16 adds of the 16 ET tiles on
    the vector engine (S), then (ones/w1)^T @ S broadcast-sums across
    partitions on the PE; linv = 1/pl then already carries w1.
  - w0 is folded into the image BN affine (relu(w0*x) = w0*relu(x));
    the exp scale compensates with 1/(w0*sqrt(C)).
  - Output written per-qb as bf16; host casts back to fp32.
"""

import math
import os
import sys
from contextlib import ExitStack

import ml_dtypes
import numpy as np

sys.path.insert(0, "/opt/trn_rl_repo")

import concourse.bass as bass  # noqa: E402
import concourse.tile as tile  # noqa: E402
from concourse import bacc, mybir  # noqa: E402
from concourse.bass import ds, ts  # noqa: E402
from concourse.bass_utils import run_bass_kernel_spmd  # noqa: E402

F32 = mybir.dt.float32
BF16 = mybir.dt.bfloat16
NPBF16 = ml_dtypes.bfloat16

B, CL, CI, CO = 2, 256, 512, 128
H = W = 128
P = H * W                    # 16384 pixels per batch
CHUNK = 2048                 # attention chunk (pixels)
NCH = P // CHUNK             # 8 chunks per batch
NCORES = 8
UPC = (B * NCH) // NCORES    # units (b,chunk) per core = 2
EPS = 1e-5
QB = 1024                    # q-block width (one 2-bank PSUM matmul)
NQB = CHUNK // QB            # 2
KSL = CHUNK // 128           # 16 k-pixel slices per chunk
NSL = NQB * KSL              # 32 (qb, slice) score tiles per unit
NCI_IMG = CI // 128          # 4 contraction slices for image proj
NCI_LID = CL // 128          # 2 for lidar proj

_PROGRAM = None              # compiled Bass program, built once per process
LAST_RESULTS = None          # BassKernelResults of the last kernel() call


def _build_program():
    nc = bacc.Bacc("TRN2", target_bir_lowering=False, debug=False,
                   num_devices=NCORES)

    # Per-core DRAM inputs (pre-sharded, bf16 on host).
    ximg = nc.dram_tensor("ximg", [UPC, NCI_IMG, 128, CHUNK], BF16,
                          kind="ExternalInput").ap()
    xlid = nc.dram_tensor("xlid", [UPC, NCI_LID, 128, CHUNK], BF16,
                          kind="ExternalInput").ap()
    wimg = nc.dram_tensor("wimg", [NCI_IMG, 128, CO], BF16,
                          kind="ExternalInput").ap()
    wlid = nc.dram_tensor("wlid", [NCI_LID, 128, CO], BF16,
                          kind="ExternalInput").ap()
    img_bias = nc.dram_tensor("img_bias", [CO, 1], F32, kind="ExternalInput").ap()
    lid_bias = nc.dram_tensor("lid_bias", [CO, 1], F32, kind="ExternalInput").ap()
    escale = nc.dram_tensor("escale", [128, 1], F32, kind="ExternalInput").ap()
    ident = nc.dram_tensor("ident", [128, 128], BF16, kind="ExternalInput").ap()
    ones_m = nc.dram_tensor("ones_m", [128, 128], BF16, kind="ExternalInput").ap()
    y = nc.dram_tensor("y", [UPC, NQB, CO, QB], BF16, kind="ExternalOutput").ap()

    with tile.TileContext(nc) as tc, ExitStack() as ctx:
        const = ctx.enter_context(tc.tile_pool(name="const", bufs=1))
        xi_pool = ctx.enter_context(tc.tile_pool(name="xi", bufs=UPC * NCI_IMG))
        xl_pool = ctx.enter_context(tc.tile_pool(name="xl", bufs=UPC * NCI_LID))
        proj_pool = ctx.enter_context(tc.tile_pool(name="proj", bufs=4))
        kp_pool = ctx.enter_context(tc.tile_pool(name="kp", bufs=4))
        et_pool = ctx.enter_context(tc.tile_pool(name="et", bufs=8))
        tree_pool = ctx.enter_context(tc.tile_pool(name="tree", bufs=10))
        misc_pool = ctx.enter_context(tc.tile_pool(name="misc", bufs=2))
        res_pool = ctx.enter_context(tc.tile_pool(name="res", bufs=2))
        # PSUM (8 banks of [128, 2KB]):
        #   mm 2x[128,1024]f32 (4 banks) - scores, double buffered; also
        #     borrowed for proj, transposes (bf16) and the lb matmul
        #   av 2x[128,1024]f32 (4 banks) - AV accumulation, double buffered
        #     so the next q-block's AV can start before the previous blend
        mm_psum = ctx.enter_context(tc.tile_pool(name="mmps", bufs=2, space="PSUM"))
        av_psum = ctx.enter_context(tc.tile_pool(name="avps", bufs=2, space="PSUM"))

        # constants
        wimg_t = const.tile([128, NCI_IMG * CO], BF16)
        for ci in range(NCI_IMG):
            nc.sync.dma_start(wimg_t[:, ts(ci, CO)], wimg[ci])
        wlid_t = const.tile([128, NCI_LID * CO], BF16)
        for ci in range(NCI_LID):
            nc.sync.dma_start(wlid_t[:, ts(ci, CO)], wlid[ci])
        img_b = const.tile([128, 1], F32)
        nc.sync.dma_start(img_b[:], img_bias)
        lid_b = const.tile([128, 1], F32)
        nc.sync.dma_start(lid_b[:], lid_bias)
        esc = const.tile([128, 1], F32)
        nc.sync.dma_start(esc[:], escale)
        ident_t = const.tile([128, 128], BF16)
        nc.sync.dma_start(ident_t[:], ident)
        ones_t = const.tile([128, 128], BF16)
        nc.sync.dma_start(ones_t[:], ones_m)

        # ---- all input DMAs up front; lidar first (the score stream needs
        # the full K projection before anything else), halves so the first
        # projection group can start as early as possible ----
        xi = {}
        xl = {}
        for u in range(UPC):
            xl[u] = [xl_pool.tile([128, CHUNK], BF16, name=f"xl_{u}_{ci}",
                                  tag="xl") for ci in range(NCI_LID)]
            xi[u] = [xi_pool.tile([128, CHUNK], BF16, name=f"xi_{u}_{ci}",
                                  tag="xi") for ci in range(NCI_IMG)]
            for hh in range(2):
                for ci in range(NCI_LID):
                    nc.sync.dma_start(xl[u][ci][:, ts(hh, QB)],
                                      xlid[u, ci, :, ts(hh, QB)])
                for ci in range(NCI_IMG):
                    nc.sync.dma_start(xi[u][ci][:, ts(hh, QB)],
                                      ximg[u, ci, :, ts(hh, QB)])

        qT = {}
        kT = {}
        kpw = {}

        def emit_proj(u):
            """Projections for unit u (channel-major; BN scale pre-folded in
            the weights, epilogue = max(psum + bias, 0) on GPSIMD) plus the
            K transposes.  Ordered so kT completes first."""
            qT[u] = proj_pool.tile([128, CHUNK], BF16, name=f"qT_{u}", tag="qT")
            kT[u] = proj_pool.tile([128, CHUNK], BF16, name=f"kT_{u}", tag="kT")
            for qb in range(NQB):
                ps = mm_psum.tile([128, QB], F32, name=f"psl_{u}_{qb}", tag="ps")
                for ci in range(NCI_LID):
                    for h in range(QB // 512):
                        nc.tensor.matmul(ps[:, ts(h, 512)], wlid_t[:, ts(ci, CO)],
                                         xl[u][ci][:, ds(qb * QB + h * 512, 512)],
                                         start=(ci == 0), stop=(ci == NCI_LID - 1))
                nc.scalar.activation(kT[u][:, ts(qb, QB)], ps[:],
                                     mybir.ActivationFunctionType.Relu,
                                     bias=lid_b[:])
            ps = mm_psum.tile([128, QB], F32, name=f"psi_{u}_0", tag="ps")
            for ci in range(NCI_IMG):
                for h in range(QB // 512):
                    nc.tensor.matmul(ps[:, ts(h, 512)], wimg_t[:, ts(ci, CO)],
                                     xi[u][ci][:, ds(h * 512, 512)],
                                     start=(ci == 0), stop=(ci == NCI_IMG - 1))
            nc.scalar.activation(qT[u][:, ds(0, QB)], ps[:],
                                 mybir.ActivationFunctionType.Relu,
                                 bias=img_b[:])
            # transpose K to pixel-major tiles (bf16 PSUM transpose, one
            # 2x-eligible GPSIMD copy per 8-slice group)
            kpw[u] = []
            for g in range(KSL // 8):
                pt = mm_psum.tile([128, 8 * 128], BF16, name=f"pt_{u}_{g}",
                                  tag="ps")
                for k in range(8):
                    nc.tensor.transpose(pt[:, ts(k, 128)],
                                        kT[u][:, ts(g * 8 + k, 128)], ident_t[:])
                kpt = kp_pool.tile([128, 8 * 128], BF16,
                                   name=f"kp_{u}_{g}", tag="kp")
                nc.vector.tensor_copy(kpt[:], pt[:])
                kpw[u].append(kpt)
            ps = mm_psum.tile([128, QB], F32, name=f"psi_{u}_1", tag="ps")
            for ci in range(NCI_IMG):
                for h in range(QB // 512):
                    nc.tensor.matmul(ps[:, ts(h, 512)], wimg_t[:, ts(ci, CO)],
                                     xi[u][ci][:, ds(QB + h * 512, 512)],
                                     start=(ci == 0), stop=(ci == NCI_IMG - 1))
            nc.scalar.activation(qT[u][:, ds(QB, QB)], ps[:],
                                 mybir.ActivationFunctionType.Relu,
                                 bias=img_b[:])

        emit_proj(0)
        for u in range(UPC):
            # ---- attention: one global slice pipeline across both q-blocks.
            # scores/exp run LOOKAHEAD slices ahead of the AV matmuls so the
            # in-order PE queue never waits on the scalar engine. ----
            LOOKAHEAD = 3
            ets = [None] * NSL
            pos = [None] * NQB
            lvl1 = [None] * (KSL // 2)
            lvl2 = [None] * (KSL // 4)
            lvl3 = {}

            def qb_epilogue(qb, l3):
                """Denominator broadcast + reciprocal + blend + store, in
                512-wide halves so the chain pipelines on the DVE."""
                pl = mm_psum.tile([128, QB], F32, name=f"pl_{u}_{qb}", tag="ps")
                for h in range(QB // 512):
                    for part in range(2):
                        nc.tensor.matmul(pl[:, ts(h, 512)], ones_t[:],
                                         l3[part][:, ts(h, 512)],
                                         start=(part == 0), stop=(part == 1))
                linv = misc_pool.tile([128, QB], F32, name=f"linv_{u}_{qb}",
                                      tag="linv")
                tmp = misc_pool.tile([128, QB], F32, name=f"tmp_{u}_{qb}",
                                     tag="tmp")
                res = res_pool.tile([128, QB], BF16, name=f"res_{u}_{qb}",
                                    tag="res")
                for h in range(QB // 512):
                    nc.vector.reciprocal_approx_fast(linv[:, ts(h, 512)],
                                                     pl[:, ts(h, 512)])
                    nc.vector.tensor_mul(tmp[:, ts(h, 512)],
                                         pos[qb][:, ts(h, 512)],
                                         linv[:, ts(h, 512)])
                    nc.vector.tensor_add(res[:, ts(h, 512)], tmp[:, ts(h, 512)],
                                         qT[u][:, ds(qb * QB + h * 512, 512)])
                    nc.sync.dma_start(y[u, qb, :, ts(h, 512)],
                                      res[:, ts(h, 512)])

            for g in range(NSL + LOOKAHEAD):
                if g < NSL:
                    qb, i = divmod(g, KSL)
                    ps = mm_psum.tile([128, QB], F32,
                                      name=f"pss_{u}_{qb}_{i}", tag="ps")
                    for h in range(QB // 512):
                        nc.tensor.matmul(ps[:, ts(h, 512)], kT[u][:, ts(i, 128)],
                                         qT[u][:, ds(qb * QB + h * 512, 512)],
                                         start=True, stop=True)
                    et = et_pool.tile([128, QB], BF16,
                                      name=f"et_{u}_{qb}_{i}", tag="et")
                    nc.scalar.activation(et[:], ps[:],
                                         mybir.ActivationFunctionType.Exp,
                                         scale=esc[:])
                    ets[g] = et
                    # binary-tree accumulation of S (all-bf16 2x DVE adds)
                    if i % 2 == 1:
                        t = tree_pool.tile([128, QB], BF16,
                                           name=f"t1_{u}_{qb}_{i}", tag="t1")
                        nc.vector.tensor_add(t[:], ets[g - 1][:], et[:])
                        lvl1[i // 2] = t
                        if i % 4 == 3:
                            t2 = tree_pool.tile([128, QB], BF16,
                                                name=f"t2_{u}_{qb}_{i}",
                                                tag="t1")
                            nc.vector.tensor_add(t2[:], lvl1[i // 2 - 1][:],
                                                 t[:])
                            lvl2[i // 4] = t2
                            if i % 8 == 7:
                                t3 = tree_pool.tile([128, QB], BF16,
                                                    name=f"t3_{u}_{qb}_{i}",
                                                    tag="t1")
                                nc.vector.tensor_add(t3[:],
                                                     lvl2[i // 4 - 1][:],
                                                     t2[:])
                                lvl3[(qb, i // 8)] = t3
                else:
                    if g == NSL and u + 1 < UPC:
                        # overlap the next unit's projections with this
                        # unit's AV drain + epilogue (scalar engine idles
                        # here regardless)
                        emit_proj(u + 1)
                j = g - LOOKAHEAD
                if j >= 0:
                    qbj, i = divmod(j, KSL)
                    if i == 0:
                        pos[qbj] = av_psum.tile([128, QB], F32,
                                                name=f"po_{u}_{qbj}", tag="av")
                    kslice = kpw[u][i // 8][:, ts(i % 8, 128)]
                    for h in range(QB // 512):
                        nc.tensor.matmul(pos[qbj][:, ts(h, 512)], kslice,
                                         ets[j][:, ts(h, 512)],
                                         start=(i == 0), stop=(i == KSL - 1))
                    if i == KSL - 1:
                        qb_epilogue(qbj, [lvl3[(qbj, 0)], lvl3[(qbj, 1)]])

    nc.compile()
    return nc


def _shard_inputs(inputs):
    """Build the 8 per-core input maps from the full input dict."""
    mw = np.asarray(inputs["modality_weights"], np.float64)
    e = np.exp(mw - mw.max())
    w = (e / e.sum()).astype(np.float64)
    w0, w1 = float(w[0]), float(w[1])

    def bn_fold(gamma, beta, mean, var, mul):
        g = np.asarray(gamma, np.float64)
        b = np.asarray(beta, np.float64)
        m = np.asarray(mean, np.float64)
        v = np.asarray(var, np.float64)
        scale = g / np.sqrt(v + EPS) * mul
        bias = (b - m * g / np.sqrt(v + EPS)) * mul
        return scale, bias.astype(np.float32).reshape(CO, 1)

    i_s, i_b = bn_fold(inputs["image_gamma"], inputs["image_beta"],
                       inputs["image_mean"], inputs["image_var"], w0)
    l_s, l_b = bn_fold(inputs["lidar_gamma"], inputs["lidar_beta"],
                       inputs["lidar_mean"], inputs["lidar_var"], 1.0)

    # weight slices with the BN scale folded in, pre-transposed for lhsT
    # ([cin_slice, cout])
    wi = np.ascontiguousarray(
        (np.asarray(inputs["image_w"], np.float64) * i_s[:, None]).T.reshape(
            NCI_IMG, 128, CO)).astype(NPBF16)
    wl = np.ascontiguousarray(
        (np.asarray(inputs["lidar_w"], np.float64) * l_s[:, None]).T.reshape(
            NCI_LID, 128, CO)).astype(NPBF16)

    esc = np.full((128, 1), 1.0 / (w0 * math.sqrt(CO)), np.float32)
    ident = np.eye(128, dtype=np.float32).astype(NPBF16)
    # ones carry 1/w1 so linv = 1/pl = w1/denominator
    ones_m = np.full((128, 128), 1.0 / w1, np.float32).astype(NPBF16)

    # full features reshaped to (B, nchunks, C, 2048), bf16
    img = np.asarray(inputs["image_features"], np.float32).reshape(
        B, CI, NCH, CHUNK).astype(NPBF16)
    lid = np.asarray(inputs["lidar_features"], np.float32).reshape(
        B, CL, NCH, CHUNK).astype(NPBF16)

    in_maps = []
    for core in range(NCORES):
        ximg = np.empty((UPC, NCI_IMG, 128, CHUNK), NPBF16)
        xlid = np.empty((UPC, NCI_LID, 128, CHUNK), NPBF16)
        for ul in range(UPC):
            un = core * UPC + ul
            b, c = un // NCH, un % NCH
            ximg[ul] = img[b, :, c, :].reshape(NCI_IMG, 128, CHUNK)
            xlid[ul] = lid[b, :, c, :].reshape(NCI_LID, 128, CHUNK)
        in_maps.append({
            "ximg": ximg, "xlid": xlid, "wimg": wi, "wlid": wl,
            "img_bias": i_b, "lid_bias": l_b,
            "escale": esc, "ident": ident, "ones_m": ones_m,
        })
    return in_maps


def kernel(**inputs) -> np.ndarray:
    global _PROGRAM, LAST_RESULTS
    if _PROGRAM is None:
        _PROGRAM = _build_program()
    nc = _PROGRAM

    in_maps = _shard_inputs(inputs)
    trace = os.environ.get("BASS_KERNEL_TRACE", "0") == "1"
    tmpdir = os.environ.get("BASS_KERNEL_TRACE_DIR") or None
    if tmpdir:
        os.makedirs(tmpdir, exist_ok=True)
    results = run_bass_kernel_spmd(nc, in_maps, core_ids=list(range(NCORES)),
                                   trace=trace, tmpdir=tmpdir)
    LAST_RESULTS = results

    out = np.empty((B, CO, H, W), np.float32)
    outv = out.reshape(B, CO, NCH, NQB, QB)
    for core in range(NCORES):
        yc = np.asarray(results.results[core]["y"], dtype=np.float32)
        for ul in range(UPC):
            un = core * UPC + ul
            b, c = un // NCH, un % NCH
            outv[b, :, c, :, :] = yc[ul].transpose(1, 0, 2)
    return out


if __name__ == "__main__":
    rng = np.random.default_rng(0)
    inputs = {
        "lidar_features": rng.standard_normal((B, CL, H, W), np.float32),
        "image_features": rng.standard_normal((B, CI, H, W), np.float32),
        "lidar_w": rng.standard_normal((CO, CL), np.float32) * np.sqrt(2.0 / CO),
        "lidar_gamma": np.ones(CO, np.float32),
        "lidar_beta": np.zeros(CO, np.float32),
        "lidar_mean": rng.standard_normal(CO).astype(np.float32) * 0.1,
        "lidar_var": rng.uniform(0.5, 1.5, CO).astype(np.float32),
        "image_w": rng.standard_normal((CO, CI), np.float32) * np.sqrt(2.0 / CO),
        "image_gamma": np.ones(CO, np.float32),
        "image_beta": np.zeros(CO, np.float32),
        "image_mean": rng.standard_normal(CO).astype(np.float32) * 0.1,
        "image_var": rng.uniform(0.5, 1.5, CO).astype(np.float32),
        "modality_weights": np.ones(2, np.float32),
    }
    out = kernel(**inputs)
    print("kernel out:", out.shape, out.dtype, float(np.abs(out).mean()))
